# revision 25
# baseline (speedup 1.0000x reference)
"""Multi-head attention (B=2, S=2048, D=1024, H=16) on 8 TRN2 NeuronCores.

Sharding (data + tensor parallel, per the head-group hint):
  core c in 0..7 -> batch b = c // 4, head-group g = c % 4 (4 heads, 256 dims).
  Each core computes, for its batch and head group:
    QT = (x @ Wq_g + bq_g)^T          [256, 2048]   (d on partitions)
    KT likewise                       [256, 2048]
    V  = x @ Wv_g                     [2048, 256]   (S on partitions; bv folded
                                                     into the host-side output
                                                     constant: softmax weights
                                                     sum to 1, so +bv passes
                                                     through attention intact)
    per head h (4 local, Dh=64):
      ST_h = K_h @ Q_h^T              [2048k, 2048q] (scores transposed)
      E_h  = exp(ST_h / 8)            (softmax without max-subtraction; scores ~ N(0,1))
      CU_h = [V_h | 1]^T @ E_h        -> ctx^T unnormalized [64, q] + row of sums s_h[q]
      CT_h = CU_h / s_h               (ctx^T, normalized)
    OT_partial = Wo_g^T @ CT          [1024, 2048]  (out^T, partial over head groups)
  Host: out[b] = (sum_g OT_partial)^T + bo + bv @ Wo.

The whole data plane is bf16 (same precision class as the PE's fp32r path,
which rounds operands to ~bf16 anyway): the host pre-converts x^T and the
weights to bf16, halving input DMA, and the fp16 output halves output DMA.
The big tensors (xt/qt/kt/ct/va/weights) are double-buffered so consecutive
reps of the replicated timing program pipeline cleanly. Host passes x
pre-transposed per batch so no on-chip transpose of x is needed.

Engine budget: the PE stream (~137us: scores run as two 64-row-group matmuls
that the PE executes concurrently, AV+projections stream-bound) is the wall;
the exp stream costs ~(N+352)/1.2 ns per ACTIVATE => ~147us for all 128
tiles, so one ki-pair per segment is computed on the (otherwise slack)
vector engine with a Schraudolph fast-exp: one fused multiply-add into an
int16 view whose bit pattern IS ~exp(x) in bf16 (max ~3% per-element error,
which averages out through the softmax normalization), leaving ACT at
~128us. Q/K biases ride the existing psum->sbuf copies as per-partition
tensor_scalar adds, so the PE runs no bias matmuls at all. A softmax-
invariant shift keeps exp values small (also required by the disabled
fp8e4m3 DoubleRow AV experiment, which cost too much accuracy to enable).
"""

import numpy as np

B = 2
S = 2048
D = 1024
DL = 256          # local (per-core) d_model slice = 4 heads * 64
HL = 4            # local heads
DH = 64
QS = 512          # q tile (matmul free dim)
NQS = S // QS     # 4
KC = 128          # k chunk (psum partitions)
NKC = S // KC     # 16
DC = 128          # contraction chunk
NDC = D // DC     # 8
NCORES = 8

# Schraudolph fast-exp constants for exp(x * 0.125) via the bf16 bit pattern
# (top 16 bits of fp32): bits = int16(x * (2^7 * log2(e) / 8) + (127*2^7 - C)),
# C centers the piecewise-linear mantissa error to ~+-3%.
EXP_SHIFT = 2.25  # exp(st/8 - 2.25): softmax-invariant shift; max score ~8 -> e^5.75=314 < 448 (fp8e4m3 max), while minimizing subnormal-crushed weight mass
                  # exp values under fp8e4m3's 448 max (raw scores reach ~8)
FEXP_A = (1 << 7) * 1.4426950408889634 * 0.125
FEXP_B = (127.0 * (1 << 7) - 250829.0 / 65536.0
          - EXP_SHIFT * 1.4426950408889634 * (1 << 7))
# ki-PAIR slots of each (pr, qs) segment whose exp runs on DVE instead of ACT
# (bf16 fast-exp + normal AV matmuls); the remaining pairs are written by ACT
# as fp8e4m3 and consumed by DoubleRow AV matmuls (2 k-chunks per pass).
# Pairs 0 and 7 must stay on the ACT/fp8 path (they carry start/stop).
DVE_PAIRS = ()
FP8_AV = False

_RUNNER = None


def _build_program(reps=1):
    import concourse.mybir as mybir
    import concourse.tile as tile
    from concourse import bacc

    F32 = mybir.dt.float32
    F32R = mybir.dt.float32r
    I16 = mybir.dt.int16
    BF16 = mybir.dt.bfloat16
    FP8 = mybir.dt.float8e4
    FP16 = mybir.dt.float16
    DoubleRow = mybir.MatmulPerfMode.DoubleRow
    Exp = mybir.ActivationFunctionType.Exp
    Mult = mybir.AluOpType.mult
    Add = mybir.AluOpType.add

    nc = bacc.Bacc("TRN2", target_bir_lowering=False, debug=False,
                   num_devices=NCORES, num_swdge_queues=4)

    XT = nc.dram_tensor("XT", [D, S], BF16, kind="ExternalInput").ap()
    WQ = nc.dram_tensor("WQ", [D, DL], BF16, kind="ExternalInput").ap()
    WK = nc.dram_tensor("WK", [D, DL], BF16, kind="ExternalInput").ap()
    WV = nc.dram_tensor("WV", [D, DL], BF16, kind="ExternalInput").ap()
    WO = nc.dram_tensor("WO", [DL, D], BF16, kind="ExternalInput").ap()
    # biases pre-shaped on host as [128, 2]: column m = bias for d-range
    # m*128..(m+1)*128 (per-partition scalars for the psum->sbuf copy)
    BQ = nc.dram_tensor("BQ", [128, 2], F32, kind="ExternalInput").ap()
    BK = nc.dram_tensor("BK", [128, 2], F32, kind="ExternalInput").ap()
    OT = nc.dram_tensor("OT", [D, S], FP16, kind="ExternalOutput").ap()

    with tile.TileContext(nc) as tc:
        with (
            tc.tile_pool(name="big", bufs=1) as big,      # long-lived tensors
            tc.tile_pool(name="exp", bufs=4) as expp,     # exp(ST) tiles
            tc.tile_pool(name="outc", bufs=3) as outc,    # out-proj copies
            tc.tile_pool(name="misc", bufs=3) as misc,    # recip rows etc
            tc.tile_pool(name="psA", bufs=2, space="PSUM") as psA,
            tc.tile_pool(name="psC", bufs=3, space="PSUM") as psC,
            tc.tile_pool(name="psO", bufs=1, space="PSUM") as psO,
        ):
            for _rep in range(reps):
                # ---- load inputs (DMA casts fp32 -> fp32r on the fly) ----
                # Inputs are spread across the HWDGE queues of the engines
                # that are idle at startup (SP / ACT / DVE) so descriptor
                # generation doesn't serialize on one queue: first matmul
                # needs wk + xt c0..3, first exp needs all of xt + wk + wq.
                # wv lands right after xt, wo (out-proj, ~100us in) last.
                bqs = big.tile([128, 2], F32, tag="bqs", bufs=2)
                bks = big.tile([128, 2], F32, tag="bks", bufs=2)
                nc.gpsimd.dma_start(out=bqs, in_=BQ)
                nc.gpsimd.dma_start(out=bks, in_=BK)
                wq = big.tile([128, NDC, DL], BF16, tag="wq", bufs=2)
                wk = big.tile([128, NDC, DL], BF16, tag="wk", bufs=2)
                wv = big.tile([128, NDC, DL], BF16, tag="wv", bufs=2)
                nc.scalar.dma_start(
                    out=wk,
                    in_=WK.rearrange("(c p) n -> p c n", p=128))
                nc.sync.dma_start(
                    out=wq,
                    in_=WQ.rearrange("(c p) n -> p c n", p=128))
                xt = big.tile([128, NDC, S], BF16, tag="xt", bufs=2)   # x^T
                xt_src = XT.rearrange("(c p) q -> p c q", p=128)
                xt_eng = (nc.sync, nc.scalar)
                for c in range(NDC):
                    xt_eng[c % 2].dma_start(out=xt[:, c, :], in_=xt_src[:, c, :])
                # wv/wo at the tail of the two HWDGE queues so they can't
                # jump ahead of the xt stream on the DMA engines; wo is
                # needed only by the out-projection (~100us in)
                nc.sync.dma_start(
                    out=wv,
                    in_=WV.rearrange("(c p) n -> p c n", p=128)
                )
                wo = big.tile([128, 2, D], BF16, tag="wo", bufs=2)
                nc.scalar.dma_start(
                    out=wo,
                    in_=WO.rearrange("(c p) n -> p c n", p=128))

                onesc_f = big.tile([128, HL, 1], BF16, tag="onesc_f", bufs=2)
                nc.vector.memset(onesc_f, 1.0)
                nbias = big.tile([128, 1], F32, tag="nbias", bufs=2)
                nc.vector.memset(nbias, -EXP_SHIFT)

                # ---- projections ----
                # Only the minimal prefix is emitted up-front; all other
                # projection work drips into the attention loop so its PE
                # time hides under the exp stream.
                qt = big.tile([128, 2, S], BF16, tag="qt", bufs=2)  # Q^T: [d(2x128), q]
                kt = big.tile([128, 2, S], BF16, tag="kt", bufs=2)
                va = big.tile([128, NKC, HL, DH + 1], BF16, tag="va", bufs=2)
                # fp8 copy of [V | 1] for the DoubleRow AV path; last dim
                # padded to 68 so the ki-pair stride (4*68 = 272B) is 16B-
                # aligned as DoubleRow weight APs require
                va8 = None
                if FP8_AV:
                    va8 = big.tile([128, NKC, HL, 68], FP8, tag="va8")
                    # DoubleRow weight reads are 16B-granular: zero the pad
                    # columns so they can never inject garbage
                    nc.gpsimd.memset(va8, 0.0)

                def emit_proj_qk_one(m, qs, w_t, bcol, dst, pool):
                    # tag shares slots with the pool's other tiles (time-disjoint)
                    p = pool.tile([128, QS], F32,
                                  tag="mm" if pool is psA else "op",
                                  name=f"pj_{m}_{qs}_{dst.tensor.name}")
                    for c in range(NDC):
                        nc.tensor.matmul(
                            p,
                            w_t[:, c, m * 128:(m + 1) * 128],
                            xt[:, c, qs * QS:(qs + 1) * QS],
                            start=(c == 0),
                            stop=(c == NDC - 1),
                        )
                    # psum->sbuf copy + bias add in one DVE tensor_scalar
                    nc.vector.tensor_scalar_add(
                        dst[:, m, qs * QS:(qs + 1) * QS], p, bcol[:, m:m + 1],
                    )

                def emit_proj_qk(m, pool):
                    # KT first: scores need all of K^T but only one q-slice of Q^T
                    for w_t, bcol, dst in ((wk, bks, kt), (wq, bqs, qt)):
                        for qs in range(NQS):
                            emit_proj_qk_one(m, qs, w_t, bcol, dst, pool)

                def emit_proj_v_one(sc, pool):
                    # V in [S, d] layout, augmented with a ones column per head
                    p = pool.tile([128, DL], F32,
                                  tag="mm" if pool is psA else "op",
                                  name=f"pv_{sc}")
                    for c in range(NDC):
                        nc.tensor.matmul(
                            p,
                            xt[:, c, sc * 128:(sc + 1) * 128],
                            wv[:, c, :],
                            start=(c == 0),
                            stop=(c == NDC - 1),
                        )
                    nc.vector.tensor_copy(
                        out=va[:, sc, :, 0:DH],
                        in_=p.rearrange("p (h d) -> p h d", h=HL),
                    )
                    nc.gpsimd.tensor_copy(
                        out=va[:, sc, :, DH:DH + 1], in_=onesc_f
                    )
                    if FP8_AV:
                        nc.vector.tensor_copy(
                            out=va8[:, sc, :, 0:DH],
                            in_=p.rearrange("p (h d) -> p h d", h=HL),
                        )
                        nc.gpsimd.tensor_copy(
                            out=va8[:, sc, :, DH:DH + 1], in_=onesc_f
                        )

                def emit_proj_qk_split(m, qs, w_t, bcol, dst):
                    # contraction split in half so the first 4 matmuls start
                    # when xt chunk 3 lands (~half the input-DMA time)
                    p1 = psA.tile([128, QS], F32, tag="mm",
                                  name=f"pjA_{m}_{qs}_{dst.tensor.name}")
                    for c in range(4):
                        nc.tensor.matmul(
                            p1, w_t[:, c, m * 128:(m + 1) * 128],
                            xt[:, c, qs * QS:(qs + 1) * QS],
                            start=(c == 0), stop=(c == 3),
                        )
                    s1 = misc.tile([128, QS], F32, tag="s1",
                                   name=f"s1_{m}_{qs}_{dst.tensor.name}")
                    # fold the bias into the first-half copy
                    nc.vector.tensor_scalar_add(s1, p1, bcol[:, m:m + 1])
                    p2 = psA.tile([128, QS], F32, tag="mm",
                                  name=f"pjB_{m}_{qs}_{dst.tensor.name}")
                    for c in range(4, NDC):
                        nc.tensor.matmul(
                            p2, w_t[:, c, m * 128:(m + 1) * 128],
                            xt[:, c, qs * QS:(qs + 1) * QS],
                            start=(c == 4), stop=(c == NDC - 1),
                        )
                    nc.vector.tensor_tensor(
                        out=dst[:, m, qs * QS:(qs + 1) * QS],
                        in0=p2, in1=s1, op=Add,
                    )

                # prefix: the minimum needed for attention (qs=0, pr=0, ki<4):
                # K^T(m0, k 0:512), Q^T(m0, q 0:512), V(s 0:128)
                emit_proj_qk_split(0, 0, wk, bks, kt)
                emit_proj_qk_split(0, 0, wq, bqs, qt)
                emit_proj_v_one(0, psO)
                # everything else drips into the attention stream below

                # ---- attention, epilogue, out-projection ----
                ct = big.tile([128, 2, S], BF16, tag="ct", bufs=2)  # ctx^T: [d(2x128), q]

                def emit_epilogue(ctx_j, pr, qs, j):
                    # normalize ctx^T by the softmax denominator (psum row 64),
                    # PE-free: DVE recip -> gpsimd partition_broadcast -> DVE mult
                    rc = misc.tile([1, QS], F32, tag="rc", name=f"rc_{qs}_{pr}_{j}")
                    nc.vector.reciprocal(out=rc, in_=ctx_j[DH:DH + 1, :])
                    bc = misc.tile([64, QS], F32, tag="bc", name=f"bc_{qs}_{pr}_{j}")
                    nc.gpsimd.partition_broadcast(bc, rc, channels=64)
                    nc.vector.tensor_tensor(
                        out=ct[j * 64:(j + 1) * 64, pr, qs * QS:(qs + 1) * QS],
                        in0=ctx_j[0:DH, :],
                        in1=bc,
                        op=Mult,
                    )

                def emit_outproj_m(qs, m):
                    p = psO.tile([128, QS], F32, tag="op", name=f"op_{qs}_{m}")
                    for c in range(2):
                        nc.tensor.matmul(
                            p,
                            wo[:, c, m * 128:(m + 1) * 128],
                            ct[:, c, qs * QS:(qs + 1) * QS],
                            start=(c == 0), stop=(c == 1),
                        )
                    o = outc.tile([128, QS], FP16, tag="o", name=f"o_{qs}_{m}")
                    nc.vector.tensor_copy(out=o, in_=p)
                    nc.sync.dma_start(
                        out=OT[m * 128:(m + 1) * 128, qs * QS:(qs + 1) * QS],
                        in_=o,
                    )

                # drip queue: closures emitted a-few-per-ki inside the attention
                # segments so their PE work overlaps the ACT-bound exp stream.
                # Order respects first-consumer: KT(m0,s) before scores reach
                # k=s*512; V(s) before PV(ki=s); QT(m0,s) before segment qs=s;
                # chunk-1 Q/K before the pr=1 phase; out-proj appended later.
                deferred = []   # (weight, closure): weight ~ PE ki-slots (x2)
                def _defer_qk(m_, qs_, w_, b_, d_, pool_, wt):
                    deferred.append((wt, (
                        lambda a, b, c, dd, p_:
                        lambda: emit_proj_qk_one(m_, a, b, c, dd, p_))
                        (qs_, w_, b_, d_, pool_)))

                def _defer_v(sc_):
                    deferred.append((2, (
                        lambda s_: lambda: emit_proj_v_one(s_, psO))(sc_)))

                # NOTE: Tile dependencies are emission-order-based — every
                # producer MUST be emitted before its first consumer. V(s) is
                # pinned at slot s-1 of segment (0,0) (not queued); KT(m0,s)
                # pops by slot s (scores need it from ki=4s); QT(m0,s) pops
                # well before segment (0,s).
                _defer_qk(0, 1, wk, bks, kt, psA, 2)
                _defer_qk(0, 2, wk, bks, kt, psA, 2)
                _defer_qk(0, 3, wk, bks, kt, psA, 2)
                _defer_qk(0, 1, wq, bqs, qt, psA, 2)
                _defer_qk(0, 2, wq, bqs, qt, psA, 2)
                _defer_qk(0, 3, wq, bqs, qt, psA, 2)
                for qs_ in range(NQS):
                    for w_, b_, d_ in ((wk, bks, kt), (wq, bqs, qt)):
                        _defer_qk(1, qs_, w_, b_, d_, psO, 4)
                drip_budget = 0
                for pr in range(2):              # head pair = (2pr, 2pr+1)
                    for qs in range(NQS):
                        ctx = [
                            psC.tile([DH + 1, QS], F32, tag="ctx",
                                     name=f"ctx_{qs}_{pr}_{j}")
                            for j in range(2)
                        ]
                        er = None
                        for ki in range(NKC):
                            st = psA.tile([128, 2 * QS], F32, tag="mm",
                                          name=f"st_{qs}_{pr}_{ki}")
                            for j in range(2):   # j: head-within-pair
                                nc.tensor.matmul(
                                    st[:, j * QS:(j + 1) * QS],
                                    kt[j * 64:(j + 1) * 64, pr, ki * KC:(ki + 1) * KC],
                                    qt[j * 64:(j + 1) * 64, pr, qs * QS:(qs + 1) * QS],
                                    start=True, stop=True,
                                )
                            t, hf = ki // 2, ki % 2
                            dve_pair = t in DVE_PAIRS
                            fp8_pair = FP8_AV and not dve_pair
                            if hf == 0:
                                # one er tile per ki-PAIR: [ki-half, head, q]
                                er = expp.tile(
                                    [128, 2, 2, QS],
                                    BF16 if not fp8_pair else FP8,
                                    tag="er", name=f"er_{qs}_{pr}_{t}")
                            if dve_pair:
                                # Schraudolph fast-exp on the vector engine:
                                # bits = int16(st*A + B); the int16 bit
                                # pattern read back as bf16 IS ~exp(st/8)
                                nc.vector.tensor_scalar(
                                    er[:, hf].bitcast(I16),
                                    st.rearrange("p (j q) -> p j q", j=2),
                                    FEXP_A, FEXP_B, Mult, Add,
                                )
                            else:
                                nc.scalar.activation(
                                    out=er[:, hf],
                                    in_=st.rearrange("p (j q) -> p j q", j=2),
                                    func=Exp, scale=0.125, bias=nbias,
                                )
                            if pr == 0 and qs == 0 and ki + 1 < NKC:
                                # pinned: V(s) one slot before PV(ki=s) reads it
                                emit_proj_v_one(ki + 1, psO)
                            if ki >= 1 or qs > 0 or pr > 0:
                                drip_budget += 2
                                while deferred and drip_budget >= deferred[0][0]:
                                    drip_budget -= deferred[0][0]
                                    deferred.pop(0)[1]()
                            if fp8_pair and hf == 1:
                                # DoubleRow: both k-chunks of the pair in one
                                # pass per head (fp8 weights [128, 2, 65])
                                for j in range(2):
                                    nc.tensor.matmul(
                                        ctx[j],
                                        va8[:, 2 * t:2 * t + 2, 2 * pr + j, 0:DH + 1],
                                        er[:, :, j, :],
                                        start=(t == 0), stop=(t == NKC // 2 - 1),
                                        perf_mode=DoubleRow,
                                    )
                            elif not fp8_pair:
                                for j in range(2):
                                    nc.tensor.matmul(
                                        ctx[j],
                                        va[:, ki, 2 * pr + j, :],
                                        er[:, hf, j, :],
                                        start=(not FP8_AV and ki == 0),
                                        stop=(not FP8_AV and ki == NKC - 1),
                                    )
                        for j in range(2):
                            emit_epilogue(ctx[j], pr, qs, j)
                        if pr == 1:
                            for m in range(8):
                                deferred.append((1, (
                                    lambda q_, m_:
                                    lambda: emit_outproj_m(q_, m_))(qs, m)))
                for _w, fn in deferred:
                    fn()

    nc.compile()
    return nc


def _shard_inputs(x, Wq, bq, Wk, bk, Wv, bv, Wo, bo):
    import ml_dtypes
    bf16 = ml_dtypes.bfloat16
    x = np.asarray(x, dtype=np.float32)
    in_maps = []
    for c in range(NCORES):
        b, g = c // 4, c % 4
        sl = slice(g * DL, (g + 1) * DL)
        in_maps.append({
            "XT": np.ascontiguousarray(x[b].T.astype(bf16)),
            "WQ": np.ascontiguousarray(np.asarray(Wq, np.float32)[:, sl].astype(bf16)),
            "WK": np.ascontiguousarray(np.asarray(Wk, np.float32)[:, sl].astype(bf16)),
            "WV": np.ascontiguousarray(np.asarray(Wv, np.float32)[:, sl].astype(bf16)),
            "WO": np.ascontiguousarray(np.asarray(Wo, np.float32)[sl, :].astype(bf16)),
            # [128, 2] columns: chunk m holds bias for d-range m*128..(m+1)*128
            "BQ": np.ascontiguousarray(
                np.asarray(bq, np.float32)[sl].reshape(2, 128).T),
            "BK": np.ascontiguousarray(
                np.asarray(bk, np.float32)[sl].reshape(2, 128).T),
        })
    return in_maps


def get_runner():
    global _RUNNER
    if _RUNNER is None:
        _RUNNER = _build_program()
    return _RUNNER


def kernel(x, Wq, bq, Wk, bk, Wv, bv, Wo, bo, **_ignored):
    from concourse.bass_utils import run_bass_kernel_spmd

    nc = get_runner()
    in_maps = _shard_inputs(x, Wq, bq, Wk, bk, Wv, bv, Wo, bo)
    res = run_bass_kernel_spmd(nc, in_maps, list(range(NCORES)))
    # bv never touches the device: softmax weights sum to 1, so V+bv adds
    # exactly bv to every normalized context row -> out += bv @ Wo.
    bias = (np.asarray(bv, np.float64) @ np.asarray(Wo, np.float64)
            + np.asarray(bo, np.float64))
    out = np.empty((B, S, D), dtype=np.float32)
    for b in range(B):
        acc = np.zeros((D, S), dtype=np.float64)
        for g in range(4):
            acc += np.asarray(res.results[4 * b + g]["OT"], np.float64)
        out[b] = (acc.T + bias).astype(np.float32)
    return out


# revision 27
# speedup vs baseline: 1.0012x; 1.0012x over previous
"""Multi-head attention (B=2, S=2048, D=1024, H=16) on 8 TRN2 NeuronCores.

Sharding (data + tensor parallel, per the head-group hint):
  core c in 0..7 -> batch b = c // 4, head-group g = c % 4 (4 heads, 256 dims).
  Each core computes, for its batch and head group:
    QT = (x @ Wq_g + bq_g)^T          [256, 2048]   (d on partitions)
    KT likewise                       [256, 2048]
    V  = x @ Wv_g                     [2048, 256]   (S on partitions; bv folded
                                                     into the host-side output
                                                     constant: softmax weights
                                                     sum to 1, so +bv passes
                                                     through attention intact)
    per head h (4 local, Dh=64):
      ST_h = K_h @ Q_h^T              [2048k, 2048q] (scores transposed)
      E_h  = exp(ST_h / 8)            (softmax without max-subtraction; scores ~ N(0,1))
      CU_h = [V_h | 1]^T @ E_h        -> ctx^T unnormalized [64, q] + row of sums s_h[q]
      CT_h = CU_h / s_h               (ctx^T, normalized)
    OT_partial = Wo_g^T @ CT          [1024, 2048]  (out^T, partial over head groups)
  Host: out[b] = (sum_g OT_partial)^T + bo + bv @ Wo.

The whole data plane is bf16 (same precision class as the PE's fp32r path,
which rounds operands to ~bf16 anyway): the host pre-converts x^T and the
weights to bf16, halving input DMA, and the fp16 output halves output DMA.
The big tensors (xt/qt/kt/ct/va/weights) are double-buffered so consecutive
reps of the replicated timing program pipeline cleanly. Host passes x
pre-transposed per batch so no on-chip transpose of x is needed.

Engine budget: the PE stream (~137us: scores run as two 64-row-group matmuls
that the PE executes concurrently, AV+projections stream-bound) is the wall;
the exp stream costs ~(N+352)/1.2 ns per ACTIVATE => ~147us for all 128
tiles, so one ki-pair per segment is computed on the (otherwise slack)
vector engine with a Schraudolph fast-exp: one fused multiply-add into an
int16 view whose bit pattern IS ~exp(x) in bf16 (max ~3% per-element error,
which averages out through the softmax normalization), leaving ACT at
~128us. Q/K biases ride the existing psum->sbuf copies as per-partition
tensor_scalar adds, so the PE runs no bias matmuls at all. A softmax-
invariant shift keeps exp values small (also required by the disabled
fp8e4m3 DoubleRow AV experiment, which cost too much accuracy to enable).
"""

import numpy as np

B = 2
S = 2048
D = 1024
DL = 256          # local (per-core) d_model slice = 4 heads * 64
HL = 4            # local heads
DH = 64
QS = 512          # q tile (matmul free dim)
NQS = S // QS     # 4
KC = 128          # k chunk (psum partitions)
NKC = S // KC     # 16
DC = 128          # contraction chunk
NDC = D // DC     # 8
NCORES = 8

# Schraudolph fast-exp constants for exp(x * 0.125) via the bf16 bit pattern
# (top 16 bits of fp32): bits = int16(x * (2^7 * log2(e) / 8) + (127*2^7 - C)),
# C centers the piecewise-linear mantissa error to ~+-3%.
EXP_SHIFT = 2.25  # exp(st/8 - 2.25): softmax-invariant shift; max score ~8 -> e^5.75=314 < 448 (fp8e4m3 max), while minimizing subnormal-crushed weight mass
                  # exp values under fp8e4m3's 448 max (raw scores reach ~8)
FEXP_A = (1 << 7) * 1.4426950408889634 * 0.125
FEXP_B = (127.0 * (1 << 7) - 250829.0 / 65536.0
          - EXP_SHIFT * 1.4426950408889634 * (1 << 7))
# ki-PAIR slots of each (pr, qs) segment whose exp runs on DVE instead of ACT
# (bf16 fast-exp + normal AV matmuls); the remaining pairs are written by ACT
# as fp8e4m3 and consumed by DoubleRow AV matmuls (2 k-chunks per pass).
# Pairs 0 and 7 must stay on the ACT/fp8 path (they carry start/stop).
DVE_PAIRS = (4,)
FP8_AV = False
# fp8e4m3 DoubleRow out-projection: ct/wo quantize safely (no exp downstream,
# values well inside fp8 range); wo scaled x16 on-device to dodge subnormals,
# host divides the partials back.
FP8_OUT = False  # measured 3.4e-2 rel err (max-statistics of e4m3 noise) - fails the gate
WO8_SCALE = 16.0

_RUNNER = None


def _build_program(reps=1):
    import concourse.mybir as mybir
    import concourse.tile as tile
    from concourse import bacc

    F32 = mybir.dt.float32
    F32R = mybir.dt.float32r
    I16 = mybir.dt.int16
    BF16 = mybir.dt.bfloat16
    FP8 = mybir.dt.float8e4
    FP16 = mybir.dt.float16
    DoubleRow = mybir.MatmulPerfMode.DoubleRow
    Exp = mybir.ActivationFunctionType.Exp
    Mult = mybir.AluOpType.mult
    Add = mybir.AluOpType.add

    nc = bacc.Bacc("TRN2", target_bir_lowering=False, debug=False,
                   num_devices=NCORES, num_swdge_queues=4)

    XT = nc.dram_tensor("XT", [D, S], BF16, kind="ExternalInput").ap()
    WQ = nc.dram_tensor("WQ", [D, DL], BF16, kind="ExternalInput").ap()
    WK = nc.dram_tensor("WK", [D, DL], BF16, kind="ExternalInput").ap()
    WV = nc.dram_tensor("WV", [D, DL], BF16, kind="ExternalInput").ap()
    WO = nc.dram_tensor("WO", [DL, D], BF16, kind="ExternalInput").ap()
    # biases pre-shaped on host as [128, 2]: column m = bias for d-range
    # m*128..(m+1)*128 (per-partition scalars for the psum->sbuf copy)
    BQ = nc.dram_tensor("BQ", [128, 2], F32, kind="ExternalInput").ap()
    BK = nc.dram_tensor("BK", [128, 2], F32, kind="ExternalInput").ap()
    OT = nc.dram_tensor("OT", [D, S], FP16, kind="ExternalOutput").ap()

    with tile.TileContext(nc) as tc:
        with (
            tc.tile_pool(name="big", bufs=1) as big,      # long-lived tensors
            tc.tile_pool(name="exp", bufs=4) as expp,     # exp(ST) tiles
            tc.tile_pool(name="outc", bufs=3) as outc,    # out-proj copies
            tc.tile_pool(name="misc", bufs=3) as misc,    # recip rows etc
            tc.tile_pool(name="psA", bufs=2, space="PSUM") as psA,
            tc.tile_pool(name="psC", bufs=3, space="PSUM") as psC,
            tc.tile_pool(name="psO", bufs=1, space="PSUM") as psO,
        ):
            for _rep in range(reps):
                # ---- load inputs (DMA casts fp32 -> fp32r on the fly) ----
                # Inputs are spread across the HWDGE queues of the engines
                # that are idle at startup (SP / ACT / DVE) so descriptor
                # generation doesn't serialize on one queue: first matmul
                # needs wk + xt c0..3, first exp needs all of xt + wk + wq.
                # wv lands right after xt, wo (out-proj, ~100us in) last.
                bqs = big.tile([128, 2], F32, tag="bqs", bufs=2)
                bks = big.tile([128, 2], F32, tag="bks", bufs=2)
                nc.gpsimd.dma_start(out=bqs, in_=BQ)
                nc.gpsimd.dma_start(out=bks, in_=BK)
                wq = big.tile([128, NDC, DL], BF16, tag="wq", bufs=2)
                wk = big.tile([128, NDC, DL], BF16, tag="wk", bufs=2)
                wv = big.tile([128, NDC, DL], BF16, tag="wv", bufs=2)
                nc.scalar.dma_start(
                    out=wk,
                    in_=WK.rearrange("(c p) n -> p c n", p=128))
                nc.sync.dma_start(
                    out=wq,
                    in_=WQ.rearrange("(c p) n -> p c n", p=128))
                xt = big.tile([128, NDC, S], BF16, tag="xt", bufs=2)   # x^T
                xt_src = XT.rearrange("(c p) q -> p c q", p=128)
                xt_eng = (nc.sync, nc.scalar)
                for c in range(NDC):
                    xt_eng[c % 2].dma_start(out=xt[:, c, :], in_=xt_src[:, c, :])
                # wv/wo at the tail of the two HWDGE queues so they can't
                # jump ahead of the xt stream on the DMA engines; wo is
                # needed only by the out-projection (~100us in)
                nc.sync.dma_start(
                    out=wv,
                    in_=WV.rearrange("(c p) n -> p c n", p=128)
                )
                wo = big.tile([128, 2, D], BF16, tag="wo", bufs=2)
                nc.scalar.dma_start(
                    out=wo,
                    in_=WO.rearrange("(c p) n -> p c n", p=128))
                wo8 = None
                if FP8_OUT:
                    wo8 = big.tile([128, 2, D], FP8, tag="wo8", bufs=2)
                    nc.gpsimd.tensor_scalar_mul(wo8, wo, WO8_SCALE)

                onesc_f = big.tile([128, HL, 1], BF16, tag="onesc_f", bufs=2)
                nc.vector.memset(onesc_f, 1.0)
                nbias = big.tile([128, 1], F32, tag="nbias", bufs=2)
                nc.vector.memset(nbias, -EXP_SHIFT)

                # ---- projections ----
                # Only the minimal prefix is emitted up-front; all other
                # projection work drips into the attention loop so its PE
                # time hides under the exp stream.
                qt = big.tile([128, 2, S], BF16, tag="qt", bufs=2)  # Q^T: [d(2x128), q]
                kt = big.tile([128, 2, S], BF16, tag="kt", bufs=2)
                va = big.tile([128, NKC, HL, DH + 1], BF16, tag="va", bufs=2)
                # fp8 copy of [V | 1] for the DoubleRow AV path; last dim
                # padded to 68 so the ki-pair stride (4*68 = 272B) is 16B-
                # aligned as DoubleRow weight APs require
                va8 = None
                if FP8_AV:
                    va8 = big.tile([128, NKC, HL, 68], FP8, tag="va8")
                    # DoubleRow weight reads are 16B-granular: zero the pad
                    # columns so they can never inject garbage
                    nc.gpsimd.memset(va8, 0.0)

                def emit_proj_qk_one(m, qs, w_t, bcol, dst, pool):
                    # tag shares slots with the pool's other tiles (time-disjoint)
                    p = pool.tile([128, QS], F32,
                                  tag="mm" if pool is psA else "op",
                                  name=f"pj_{m}_{qs}_{dst.tensor.name}")
                    for c in range(NDC):
                        nc.tensor.matmul(
                            p,
                            w_t[:, c, m * 128:(m + 1) * 128],
                            xt[:, c, qs * QS:(qs + 1) * QS],
                            start=(c == 0),
                            stop=(c == NDC - 1),
                        )
                    # psum->sbuf copy + bias add in one DVE tensor_scalar
                    nc.vector.tensor_scalar_add(
                        dst[:, m, qs * QS:(qs + 1) * QS], p, bcol[:, m:m + 1],
                    )

                def emit_proj_qk(m, pool):
                    # KT first: scores need all of K^T but only one q-slice of Q^T
                    for w_t, bcol, dst in ((wk, bks, kt), (wq, bqs, qt)):
                        for qs in range(NQS):
                            emit_proj_qk_one(m, qs, w_t, bcol, dst, pool)

                def emit_proj_v_one(sc, pool):
                    # V in [S, d] layout, augmented with a ones column per head
                    p = pool.tile([128, DL], F32,
                                  tag="mm" if pool is psA else "op",
                                  name=f"pv_{sc}")
                    for c in range(NDC):
                        nc.tensor.matmul(
                            p,
                            xt[:, c, sc * 128:(sc + 1) * 128],
                            wv[:, c, :],
                            start=(c == 0),
                            stop=(c == NDC - 1),
                        )
                    nc.vector.tensor_copy(
                        out=va[:, sc, :, 0:DH],
                        in_=p.rearrange("p (h d) -> p h d", h=HL),
                    )
                    nc.gpsimd.tensor_copy(
                        out=va[:, sc, :, DH:DH + 1], in_=onesc_f
                    )
                    if FP8_AV:
                        nc.vector.tensor_copy(
                            out=va8[:, sc, :, 0:DH],
                            in_=p.rearrange("p (h d) -> p h d", h=HL),
                        )
                        nc.gpsimd.tensor_copy(
                            out=va8[:, sc, :, DH:DH + 1], in_=onesc_f
                        )

                def emit_proj_qk_split(m, qs, w_t, bcol, dst):
                    # contraction split in half so the first 4 matmuls start
                    # when xt chunk 3 lands (~half the input-DMA time)
                    p1 = psA.tile([128, QS], F32, tag="mm",
                                  name=f"pjA_{m}_{qs}_{dst.tensor.name}")
                    for c in range(4):
                        nc.tensor.matmul(
                            p1, w_t[:, c, m * 128:(m + 1) * 128],
                            xt[:, c, qs * QS:(qs + 1) * QS],
                            start=(c == 0), stop=(c == 3),
                        )
                    s1 = misc.tile([128, QS], F32, tag="s1",
                                   name=f"s1_{m}_{qs}_{dst.tensor.name}")
                    # fold the bias into the first-half copy
                    nc.vector.tensor_scalar_add(s1, p1, bcol[:, m:m + 1])
                    p2 = psA.tile([128, QS], F32, tag="mm",
                                  name=f"pjB_{m}_{qs}_{dst.tensor.name}")
                    for c in range(4, NDC):
                        nc.tensor.matmul(
                            p2, w_t[:, c, m * 128:(m + 1) * 128],
                            xt[:, c, qs * QS:(qs + 1) * QS],
                            start=(c == 4), stop=(c == NDC - 1),
                        )
                    nc.vector.tensor_tensor(
                        out=dst[:, m, qs * QS:(qs + 1) * QS],
                        in0=p2, in1=s1, op=Add,
                    )

                # prefix: the minimum needed for attention (qs=0, pr=0, ki<4):
                # K^T(m0, k 0:512), Q^T(m0, q 0:512), V(s 0:128)
                emit_proj_qk_split(0, 0, wk, bks, kt)
                emit_proj_qk_split(0, 0, wq, bqs, qt)
                emit_proj_v_one(0, psO)
                # everything else drips into the attention stream below

                # ---- attention, epilogue, out-projection ----
                ct = big.tile([128, 2, S], FP8 if FP8_OUT else BF16,
                              tag="ct", bufs=2)   # ctx^T: [d(2x128), q]

                def emit_epilogue(ctx_j, pr, qs, j):
                    # normalize ctx^T by the softmax denominator (psum row 64),
                    # PE-free: DVE recip -> gpsimd partition_broadcast -> DVE mult
                    rc = misc.tile([1, QS], F32, tag="rc", name=f"rc_{qs}_{pr}_{j}")
                    nc.vector.reciprocal(out=rc, in_=ctx_j[DH:DH + 1, :])
                    bc = misc.tile([64, QS], F32, tag="bc", name=f"bc_{qs}_{pr}_{j}")
                    nc.gpsimd.partition_broadcast(bc, rc, channels=64)
                    nc.vector.tensor_tensor(
                        out=ct[j * 64:(j + 1) * 64, pr, qs * QS:(qs + 1) * QS],
                        in0=ctx_j[0:DH, :],
                        in1=bc,
                        op=Mult,
                    )

                def emit_outproj_m(qs, m, tail=False):
                    p = psO.tile([128, QS], F32, tag="op", name=f"op_{qs}_{m}")
                    if FP8_OUT:
                        # both 128-row contraction chunks in one DoubleRow pass
                        nc.tensor.matmul(
                            p,
                            wo8[:, :, m * 128:(m + 1) * 128],
                            ct[:, :, qs * QS:(qs + 1) * QS],
                            start=True, stop=True,
                            perf_mode=DoubleRow,
                        )
                    else:
                        for c in range(2):
                            nc.tensor.matmul(
                                p,
                                wo[:, c, m * 128:(m + 1) * 128],
                                ct[:, c, qs * QS:(qs + 1) * QS],
                                start=(c == 0), stop=(c == 1),
                            )
                    o = outc.tile([128, QS], FP16, tag="o", name=f"o_{qs}_{m}")
                    if tail:
                        # rep tail: ACT is idle after the last exp; putting the
                        # psum->sbuf copies there frees the psO slot without
                        # waiting on the DVE queue
                        nc.scalar.copy(out=o, in_=p)
                    else:
                        nc.vector.tensor_copy(out=o, in_=p)
                    nc.sync.dma_start(
                        out=OT[m * 128:(m + 1) * 128, qs * QS:(qs + 1) * QS],
                        in_=o,
                    )

                # drip queue: closures emitted a-few-per-ki inside the attention
                # segments so their PE work overlaps the ACT-bound exp stream.
                # Order respects first-consumer: KT(m0,s) before scores reach
                # k=s*512; V(s) before PV(ki=s); QT(m0,s) before segment qs=s;
                # chunk-1 Q/K before the pr=1 phase; out-proj appended later.
                deferred = []   # (weight, closure): weight ~ PE ki-slots (x2)
                def _defer_qk(m_, qs_, w_, b_, d_, pool_, wt):
                    deferred.append((wt, (
                        lambda a, b, c, dd, p_:
                        lambda: emit_proj_qk_one(m_, a, b, c, dd, p_))
                        (qs_, w_, b_, d_, pool_)))

                def _defer_v(sc_):
                    deferred.append((2, (
                        lambda s_: lambda: emit_proj_v_one(s_, psO))(sc_)))

                # NOTE: Tile dependencies are emission-order-based — every
                # producer MUST be emitted before its first consumer. V(s) is
                # pinned at slot s-1 of segment (0,0) (not queued); KT(m0,s)
                # pops by slot s (scores need it from ki=4s); QT(m0,s) pops
                # well before segment (0,s).
                _defer_qk(0, 1, wk, bks, kt, psA, 2)
                _defer_qk(0, 2, wk, bks, kt, psA, 2)
                _defer_qk(0, 3, wk, bks, kt, psA, 2)
                _defer_qk(0, 1, wq, bqs, qt, psA, 2)
                _defer_qk(0, 2, wq, bqs, qt, psA, 2)
                _defer_qk(0, 3, wq, bqs, qt, psA, 2)
                for qs_ in range(NQS):
                    for w_, b_, d_ in ((wk, bks, kt), (wq, bqs, qt)):
                        _defer_qk(1, qs_, w_, b_, d_, psO, 4)
                drip_budget = 0
                for pr in range(2):              # head pair = (2pr, 2pr+1)
                    for qs in range(NQS):
                        ctx = [
                            psC.tile([DH + 1, QS], F32, tag="ctx",
                                     name=f"ctx_{qs}_{pr}_{j}")
                            for j in range(2)
                        ]
                        er = None
                        for ki in range(NKC):
                            st = psA.tile([128, 2 * QS], F32, tag="mm",
                                          name=f"st_{qs}_{pr}_{ki}")
                            for j in range(2):   # j: head-within-pair
                                nc.tensor.matmul(
                                    st[:, j * QS:(j + 1) * QS],
                                    kt[j * 64:(j + 1) * 64, pr, ki * KC:(ki + 1) * KC],
                                    qt[j * 64:(j + 1) * 64, pr, qs * QS:(qs + 1) * QS],
                                    start=True, stop=True,
                                )
                            t, hf = ki // 2, ki % 2
                            dve_pair = t in DVE_PAIRS
                            fp8_pair = FP8_AV and not dve_pair
                            if hf == 0:
                                # one er tile per ki-PAIR: [ki-half, head, q]
                                er = expp.tile(
                                    [128, 2, 2, QS],
                                    BF16 if not fp8_pair else FP8,
                                    tag="er", name=f"er_{qs}_{pr}_{t}")
                            if dve_pair:
                                # Schraudolph fast-exp on the vector engine:
                                # bits = int16(st*A + B); the int16 bit
                                # pattern read back as bf16 IS ~exp(st/8)
                                nc.vector.tensor_scalar(
                                    er[:, hf].bitcast(I16),
                                    st.rearrange("p (j q) -> p j q", j=2),
                                    FEXP_A, FEXP_B, Mult, Add,
                                )
                            else:
                                nc.scalar.activation(
                                    out=er[:, hf],
                                    in_=st.rearrange("p (j q) -> p j q", j=2),
                                    func=Exp, scale=0.125, bias=nbias,
                                )
                            if pr == 0 and qs == 0 and ki + 1 < NKC:
                                # pinned: V(s) one slot before PV(ki=s) reads it
                                emit_proj_v_one(ki + 1, psO)
                            if ki >= 1 or qs > 0 or pr > 0:
                                drip_budget += 2
                                while deferred and drip_budget >= deferred[0][0]:
                                    drip_budget -= deferred[0][0]
                                    deferred.pop(0)[1]()
                            if fp8_pair and hf == 1:
                                # DoubleRow: both k-chunks of the pair in one
                                # pass per head (fp8 weights [128, 2, 65])
                                for j in range(2):
                                    nc.tensor.matmul(
                                        ctx[j],
                                        va8[:, 2 * t:2 * t + 2, 2 * pr + j, 0:DH + 1],
                                        er[:, :, j, :],
                                        start=(t == 0), stop=(t == NKC // 2 - 1),
                                        perf_mode=DoubleRow,
                                    )
                            elif not fp8_pair:
                                for j in range(2):
                                    nc.tensor.matmul(
                                        ctx[j],
                                        va[:, ki, 2 * pr + j, :],
                                        er[:, hf, j, :],
                                        start=(not FP8_AV and ki == 0),
                                        stop=(not FP8_AV and ki == NKC - 1),
                                    )
                        for j in range(2):
                            emit_epilogue(ctx[j], pr, qs, j)
                        if pr == 1:
                            for m in range(8):
                                deferred.append((1, (
                                    lambda q_, m_, t_:
                                    lambda: emit_outproj_m(q_, m_, t_))
                                    (qs, m, qs == NQS - 1)))
                for _w, fn in deferred:
                    fn()

    nc.compile()
    return nc


def _shard_inputs(x, Wq, bq, Wk, bk, Wv, bv, Wo, bo):
    import ml_dtypes
    bf16 = ml_dtypes.bfloat16
    x = np.asarray(x, dtype=np.float32)
    in_maps = []
    for c in range(NCORES):
        b, g = c // 4, c % 4
        sl = slice(g * DL, (g + 1) * DL)
        in_maps.append({
            "XT": np.ascontiguousarray(x[b].T.astype(bf16)),
            "WQ": np.ascontiguousarray(np.asarray(Wq, np.float32)[:, sl].astype(bf16)),
            "WK": np.ascontiguousarray(np.asarray(Wk, np.float32)[:, sl].astype(bf16)),
            "WV": np.ascontiguousarray(np.asarray(Wv, np.float32)[:, sl].astype(bf16)),
            "WO": np.ascontiguousarray(np.asarray(Wo, np.float32)[sl, :].astype(bf16)),
            # [128, 2] columns: chunk m holds bias for d-range m*128..(m+1)*128
            "BQ": np.ascontiguousarray(
                np.asarray(bq, np.float32)[sl].reshape(2, 128).T),
            "BK": np.ascontiguousarray(
                np.asarray(bk, np.float32)[sl].reshape(2, 128).T),
        })
    return in_maps


def get_runner():
    global _RUNNER
    if _RUNNER is None:
        _RUNNER = _build_program()
    return _RUNNER


def kernel(x, Wq, bq, Wk, bk, Wv, bv, Wo, bo, **_ignored):
    from concourse.bass_utils import run_bass_kernel_spmd

    nc = get_runner()
    in_maps = _shard_inputs(x, Wq, bq, Wk, bk, Wv, bv, Wo, bo)
    res = run_bass_kernel_spmd(nc, in_maps, list(range(NCORES)))
    # bv never touches the device: softmax weights sum to 1, so V+bv adds
    # exactly bv to every normalized context row -> out += bv @ Wo.
    bias = (np.asarray(bv, np.float64) @ np.asarray(Wo, np.float64)
            + np.asarray(bo, np.float64))
    scale = 1.0 / WO8_SCALE if FP8_OUT else 1.0
    out = np.empty((B, S, D), dtype=np.float32)
    for b in range(B):
        acc = np.zeros((D, S), dtype=np.float64)
        for g in range(4):
            acc += np.asarray(res.results[4 * b + g]["OT"], np.float64)
        out[b] = (acc.T * scale + bias).astype(np.float32)
    return out


# revision 28
# speedup vs baseline: 1.0257x; 1.0245x over previous
"""Multi-head attention (B=2, S=2048, D=1024, H=16) on 8 TRN2 NeuronCores.

Sharding (data + tensor parallel, per the head-group hint):
  core c in 0..7 -> batch b = c // 4, head-group g = c % 4 (4 heads, 256 dims).
  Each core computes, for its batch and head group:
    QT = (x @ Wq_g + bq_g)^T          [256, 2048]   (d on partitions)
    KT likewise                       [256, 2048]
    V  = x @ Wv_g                     [2048, 256]   (S on partitions; bv folded
                                                     into the host-side output
                                                     constant: softmax weights
                                                     sum to 1, so +bv passes
                                                     through attention intact)
    per head h (4 local, Dh=64):
      ST_h = K_h @ Q_h^T              [2048k, 2048q] (scores transposed)
      E_h  = exp(ST_h / 8)            (softmax without max-subtraction; scores ~ N(0,1))
      CU_h = [V_h | 1]^T @ E_h        -> ctx^T unnormalized [64, q] + row of sums s_h[q]
      CT_h = CU_h / s_h               (ctx^T, normalized)
    OT_partial = Wo_g^T @ CT          [1024, 2048]  (out^T, partial over head groups)
  Host: out[b] = (sum_g OT_partial)^T + bo + bv @ Wo.

The whole data plane is bf16 (same precision class as the PE's fp32r path,
which rounds operands to ~bf16 anyway): the host pre-converts x^T and the
weights to bf16, halving input DMA, and the fp16 output halves output DMA.
The big tensors (xt/qt/kt/ct/va/weights) are double-buffered so consecutive
reps of the replicated timing program pipeline cleanly. Host passes x
pre-transposed per batch so no on-chip transpose of x is needed.

Engine budget: the PE stream (~137us: scores run as two 64-row-group matmuls
that the PE executes concurrently, AV+projections stream-bound) is the wall;
the exp stream costs ~(N+352)/1.2 ns per ACTIVATE => ~147us for all 128
tiles, so one ki-pair per segment is computed on the (otherwise slack)
vector engine with a Schraudolph fast-exp: one fused multiply-add into an
int16 view whose bit pattern IS ~exp(x) in bf16 (max ~3% per-element error,
which averages out through the softmax normalization), leaving ACT at
~128us. Q/K biases ride the existing psum->sbuf copies as per-partition
tensor_scalar adds, so the PE runs no bias matmuls at all. A softmax-
invariant shift keeps exp values small (also required by the disabled
fp8e4m3 DoubleRow AV experiment, which cost too much accuracy to enable).
"""

import numpy as np

B = 2
S = 2048
D = 1024
DL = 256          # local (per-core) d_model slice = 4 heads * 64
HL = 4            # local heads
DH = 64
QS = 512          # q tile (matmul free dim)
NQS = S // QS     # 4
KC = 128          # k chunk (psum partitions)
NKC = S // KC     # 16
DC = 128          # contraction chunk
NDC = D // DC     # 8
NCORES = 8

# Schraudolph fast-exp constants for exp(x * 0.125) via the bf16 bit pattern
# (top 16 bits of fp32): bits = int16(x * (2^7 * log2(e) / 8) + (127*2^7 - C)),
# C centers the piecewise-linear mantissa error to ~+-3%.
EXP_SHIFT = 2.25  # exp(st/8 - 2.25): softmax-invariant shift; max score ~8 -> e^5.75=314 < 448 (fp8e4m3 max), while minimizing subnormal-crushed weight mass
                  # exp values under fp8e4m3's 448 max (raw scores reach ~8)
FEXP_A = (1 << 7) * 1.4426950408889634 * 0.125
FEXP_B = (127.0 * (1 << 7) - 250829.0 / 65536.0
          - EXP_SHIFT * 1.4426950408889634 * (1 << 7))
# ki-PAIR slots of each (pr, qs) segment whose exp runs on DVE instead of ACT
# (bf16 fast-exp + normal AV matmuls); the remaining pairs are written by ACT
# as fp8e4m3 and consumed by DoubleRow AV matmuls (2 k-chunks per pass).
# Pairs 0 and 7 must stay on the ACT/fp8 path (they carry start/stop).
DVE_PAIRS = (4,)
FP8_AV = False
# fp8e4m3 DoubleRow out-projection: ct/wo quantize safely (no exp downstream,
# values well inside fp8 range); wo scaled x16 on-device to dodge subnormals,
# host divides the partials back.
FP8_OUT = False  # measured 3.4e-2 rel err (max-statistics of e4m3 noise) - fails the gate
WO8_SCALE = 16.0

_RUNNER = None


def _build_program(reps=1):
    import concourse.mybir as mybir
    import concourse.tile as tile
    from concourse import bacc

    F32 = mybir.dt.float32
    F32R = mybir.dt.float32r
    I16 = mybir.dt.int16
    BF16 = mybir.dt.bfloat16
    FP8 = mybir.dt.float8e4
    FP16 = mybir.dt.float16
    DoubleRow = mybir.MatmulPerfMode.DoubleRow
    Exp = mybir.ActivationFunctionType.Exp
    Mult = mybir.AluOpType.mult
    Add = mybir.AluOpType.add

    nc = bacc.Bacc("TRN2", target_bir_lowering=False, debug=False,
                   num_devices=NCORES, num_swdge_queues=4)

    XT = nc.dram_tensor("XT", [D, S], BF16, kind="ExternalInput").ap()
    WQ = nc.dram_tensor("WQ", [D, DL], BF16, kind="ExternalInput").ap()
    WK = nc.dram_tensor("WK", [D, DL], BF16, kind="ExternalInput").ap()
    WV = nc.dram_tensor("WV", [D, DL], BF16, kind="ExternalInput").ap()
    WO = nc.dram_tensor("WO", [DL, D], BF16, kind="ExternalInput").ap()
    # biases pre-shaped on host as [128, 2]: column m = bias for d-range
    # m*128..(m+1)*128 (per-partition scalars for the psum->sbuf copy)
    BQ = nc.dram_tensor("BQ", [128, 2], F32, kind="ExternalInput").ap()
    BK = nc.dram_tensor("BK", [128, 2], F32, kind="ExternalInput").ap()
    OT = nc.dram_tensor("OT", [D, S], FP16, kind="ExternalOutput").ap()

    with tile.TileContext(nc) as tc:
        with (
            tc.tile_pool(name="big", bufs=1) as big,      # long-lived tensors
            tc.tile_pool(name="exp", bufs=4) as expp,     # exp(ST) tiles
            tc.tile_pool(name="outc", bufs=3) as outc,    # out-proj copies
            tc.tile_pool(name="misc", bufs=3) as misc,    # recip rows etc
            tc.tile_pool(name="psA", bufs=2, space="PSUM") as psA,
            tc.tile_pool(name="psC", bufs=3, space="PSUM") as psC,
            tc.tile_pool(name="psO", bufs=1, space="PSUM") as psO,
        ):
            for _rep in range(reps):
                # ---- load inputs (DMA casts fp32 -> fp32r on the fly) ----
                # Inputs are spread across the HWDGE queues of the engines
                # that are idle at startup (SP / ACT / DVE) so descriptor
                # generation doesn't serialize on one queue: first matmul
                # needs wk + xt c0..3, first exp needs all of xt + wk + wq.
                # wv lands right after xt, wo (out-proj, ~100us in) last.
                bqs = big.tile([128, 2], F32, tag="bqs", bufs=2)
                bks = big.tile([128, 2], F32, tag="bks", bufs=2)
                nc.gpsimd.dma_start(out=bqs, in_=BQ)
                nc.gpsimd.dma_start(out=bks, in_=BK)
                wq = big.tile([128, NDC, DL], BF16, tag="wq", bufs=2)
                wk = big.tile([128, NDC, DL], BF16, tag="wk", bufs=2)
                wv = big.tile([128, NDC, DL], BF16, tag="wv", bufs=2)
                nc.scalar.dma_start(
                    out=wk,
                    in_=WK.rearrange("(c p) n -> p c n", p=128))
                nc.sync.dma_start(
                    out=wq,
                    in_=WQ.rearrange("(c p) n -> p c n", p=128))
                xt = big.tile([128, NDC, S], BF16, tag="xt", bufs=2)   # x^T
                xt_src = XT.rearrange("(c p) q -> p c q", p=128)
                xt_eng = (nc.sync, nc.scalar)
                for c in range(NDC):
                    xt_eng[c % 2].dma_start(out=xt[:, c, :], in_=xt_src[:, c, :])
                # wv/wo at the tail of the two HWDGE queues so they can't
                # jump ahead of the xt stream on the DMA engines; wo is
                # needed only by the out-projection (~100us in)
                nc.sync.dma_start(
                    out=wv,
                    in_=WV.rearrange("(c p) n -> p c n", p=128)
                )
                wo = big.tile([128, 2, D], BF16, tag="wo", bufs=2)
                nc.scalar.dma_start(
                    out=wo,
                    in_=WO.rearrange("(c p) n -> p c n", p=128))
                wo8 = None
                if FP8_OUT:
                    wo8 = big.tile([128, 2, D], FP8, tag="wo8", bufs=2)
                    nc.gpsimd.tensor_scalar_mul(wo8, wo, WO8_SCALE)

                onesc_f = big.tile([128, HL, 1], BF16, tag="onesc_f", bufs=2)
                nc.vector.memset(onesc_f, 1.0)
                nbias = big.tile([128, 1], F32, tag="nbias", bufs=2)
                nc.vector.memset(nbias, -EXP_SHIFT)

                # ---- projections ----
                # Only the minimal prefix is emitted up-front; all other
                # projection work drips into the attention loop so its PE
                # time hides under the exp stream.
                qt = big.tile([128, 2, S], BF16, tag="qt", bufs=2)  # Q^T: [d(2x128), q]
                kt = big.tile([128, 2, S], BF16, tag="kt", bufs=2)
                va = big.tile([128, NKC, HL, DH + 1], BF16, tag="va", bufs=2)
                # the softmax-sum ones column, for all k-chunks at once
                nc.gpsimd.memset(va[:, :, :, DH:DH + 1], 1.0)
                # fp8 copy of [V | 1] for the DoubleRow AV path; last dim
                # padded to 68 so the ki-pair stride (4*68 = 272B) is 16B-
                # aligned as DoubleRow weight APs require
                va8 = None
                if FP8_AV:
                    va8 = big.tile([128, NKC, HL, 68], FP8, tag="va8")
                    # DoubleRow weight reads are 16B-granular: zero the pad
                    # columns so they can never inject garbage
                    nc.gpsimd.memset(va8, 0.0)

                def emit_proj_qk_one(m, qs, w_t, bcol, dst, pool):
                    # tag shares slots with the pool's other tiles (time-disjoint)
                    p = pool.tile([128, QS], F32,
                                  tag="mm" if pool is psA else "op",
                                  name=f"pj_{m}_{qs}_{dst.tensor.name}")
                    for c in range(NDC):
                        nc.tensor.matmul(
                            p,
                            w_t[:, c, m * 128:(m + 1) * 128],
                            xt[:, c, qs * QS:(qs + 1) * QS],
                            start=(c == 0),
                            stop=(c == NDC - 1),
                        )
                    # psum->sbuf copy + bias add in one DVE tensor_scalar
                    nc.vector.tensor_scalar_add(
                        dst[:, m, qs * QS:(qs + 1) * QS], p, bcol[:, m:m + 1],
                    )

                def emit_proj_qk(m, pool):
                    # KT first: scores need all of K^T but only one q-slice of Q^T
                    for w_t, bcol, dst in ((wk, bks, kt), (wq, bqs, qt)):
                        for qs in range(NQS):
                            emit_proj_qk_one(m, qs, w_t, bcol, dst, pool)

                def emit_proj_v_one(sc, pool):
                    # V in [S, d] layout, augmented with a ones column per head
                    p = pool.tile([128, DL], F32,
                                  tag="mm" if pool is psA else "op",
                                  name=f"pv_{sc}")
                    for c in range(NDC):
                        nc.tensor.matmul(
                            p,
                            xt[:, c, sc * 128:(sc + 1) * 128],
                            wv[:, c, :],
                            start=(c == 0),
                            stop=(c == NDC - 1),
                        )
                    nc.vector.tensor_copy(
                        out=va[:, sc, :, 0:DH],
                        in_=p.rearrange("p (h d) -> p h d", h=HL),
                    )
                    if FP8_AV:
                        nc.vector.tensor_copy(
                            out=va8[:, sc, :, 0:DH],
                            in_=p.rearrange("p (h d) -> p h d", h=HL),
                        )
                        nc.gpsimd.tensor_copy(
                            out=va8[:, sc, :, DH:DH + 1], in_=onesc_f
                        )

                def emit_proj_qk_split(m, qs, w_t, bcol, dst):
                    # contraction split in half so the first 4 matmuls start
                    # when xt chunk 3 lands (~half the input-DMA time)
                    p1 = psA.tile([128, QS], F32, tag="mm",
                                  name=f"pjA_{m}_{qs}_{dst.tensor.name}")
                    for c in range(4):
                        nc.tensor.matmul(
                            p1, w_t[:, c, m * 128:(m + 1) * 128],
                            xt[:, c, qs * QS:(qs + 1) * QS],
                            start=(c == 0), stop=(c == 3),
                        )
                    s1 = misc.tile([128, QS], F32, tag="s1",
                                   name=f"s1_{m}_{qs}_{dst.tensor.name}")
                    # fold the bias into the first-half copy
                    nc.vector.tensor_scalar_add(s1, p1, bcol[:, m:m + 1])
                    p2 = psA.tile([128, QS], F32, tag="mm",
                                  name=f"pjB_{m}_{qs}_{dst.tensor.name}")
                    for c in range(4, NDC):
                        nc.tensor.matmul(
                            p2, w_t[:, c, m * 128:(m + 1) * 128],
                            xt[:, c, qs * QS:(qs + 1) * QS],
                            start=(c == 4), stop=(c == NDC - 1),
                        )
                    nc.vector.tensor_tensor(
                        out=dst[:, m, qs * QS:(qs + 1) * QS],
                        in0=p2, in1=s1, op=Add,
                    )

                # prefix: the minimum needed for attention (qs=0, pr=0, ki<4):
                # K^T(m0, k 0:512), Q^T(m0, q 0:512), V(s 0:128)
                emit_proj_qk_split(0, 0, wk, bks, kt)
                emit_proj_qk_split(0, 0, wq, bqs, qt)
                emit_proj_v_one(0, psO)
                # everything else drips into the attention stream below

                # ---- attention, epilogue, out-projection ----
                ct = big.tile([128, 2, S], FP8 if FP8_OUT else BF16,
                              tag="ct", bufs=2)   # ctx^T: [d(2x128), q]

                def emit_epilogue(ctx_j, pr, qs, j):
                    # normalize ctx^T by the softmax denominator (psum row 64),
                    # PE-free: DVE recip -> gpsimd partition_broadcast -> DVE mult
                    rc = misc.tile([1, QS], F32, tag="rc", name=f"rc_{qs}_{pr}_{j}")
                    nc.vector.reciprocal(out=rc, in_=ctx_j[DH:DH + 1, :])
                    bc = misc.tile([64, QS], F32, tag="bc", name=f"bc_{qs}_{pr}_{j}")
                    nc.gpsimd.partition_broadcast(bc, rc, channels=64)
                    nc.vector.tensor_tensor(
                        out=ct[j * 64:(j + 1) * 64, pr, qs * QS:(qs + 1) * QS],
                        in0=ctx_j[0:DH, :],
                        in1=bc,
                        op=Mult,
                    )

                def emit_outproj_m(qs, m, tail=False):
                    p = psO.tile([128, QS], F32, tag="op", name=f"op_{qs}_{m}")
                    if FP8_OUT:
                        # both 128-row contraction chunks in one DoubleRow pass
                        nc.tensor.matmul(
                            p,
                            wo8[:, :, m * 128:(m + 1) * 128],
                            ct[:, :, qs * QS:(qs + 1) * QS],
                            start=True, stop=True,
                            perf_mode=DoubleRow,
                        )
                    else:
                        for c in range(2):
                            nc.tensor.matmul(
                                p,
                                wo[:, c, m * 128:(m + 1) * 128],
                                ct[:, c, qs * QS:(qs + 1) * QS],
                                start=(c == 0), stop=(c == 1),
                            )
                    o = outc.tile([128, QS], FP16, tag="o", name=f"o_{qs}_{m}")
                    if tail:
                        # rep tail: ACT is idle after the last exp; putting the
                        # psum->sbuf copies there frees the psO slot without
                        # waiting on the DVE queue
                        nc.scalar.copy(out=o, in_=p)
                    else:
                        nc.vector.tensor_copy(out=o, in_=p)
                    nc.sync.dma_start(
                        out=OT[m * 128:(m + 1) * 128, qs * QS:(qs + 1) * QS],
                        in_=o,
                    )

                # drip queue: closures emitted a-few-per-ki inside the attention
                # segments so their PE work overlaps the ACT-bound exp stream.
                # Order respects first-consumer: KT(m0,s) before scores reach
                # k=s*512; V(s) before PV(ki=s); QT(m0,s) before segment qs=s;
                # chunk-1 Q/K before the pr=1 phase; out-proj appended later.
                deferred = []   # (weight, closure): weight ~ PE ki-slots (x2)
                def _defer_qk(m_, qs_, w_, b_, d_, pool_, wt):
                    deferred.append((wt, (
                        lambda a, b, c, dd, p_:
                        lambda: emit_proj_qk_one(m_, a, b, c, dd, p_))
                        (qs_, w_, b_, d_, pool_)))

                def _defer_v(sc_):
                    deferred.append((2, (
                        lambda s_: lambda: emit_proj_v_one(s_, psO))(sc_)))

                # NOTE: Tile dependencies are emission-order-based — every
                # producer MUST be emitted before its first consumer. V(s) is
                # pinned at slot s-1 of segment (0,0) (not queued); KT(m0,s)
                # pops by slot s (scores need it from ki=4s); QT(m0,s) pops
                # well before segment (0,s).
                _defer_qk(0, 1, wk, bks, kt, psA, 2)
                _defer_qk(0, 2, wk, bks, kt, psA, 2)
                _defer_qk(0, 3, wk, bks, kt, psA, 2)
                _defer_qk(0, 1, wq, bqs, qt, psA, 2)
                _defer_qk(0, 2, wq, bqs, qt, psA, 2)
                _defer_qk(0, 3, wq, bqs, qt, psA, 2)
                for qs_ in range(NQS):
                    for w_, b_, d_ in ((wk, bks, kt), (wq, bqs, qt)):
                        _defer_qk(1, qs_, w_, b_, d_, psO, 4)
                drip_budget = 0
                for pr in range(2):              # head pair = (2pr, 2pr+1)
                    for qs in range(NQS):
                        ctx = [
                            psC.tile([DH + 1, QS], F32, tag="ctx",
                                     name=f"ctx_{qs}_{pr}_{j}")
                            for j in range(2)
                        ]
                        er = None
                        for ki in range(NKC):
                            st = psA.tile([128, 2 * QS], F32, tag="mm",
                                          name=f"st_{qs}_{pr}_{ki}")
                            for j in range(2):   # j: head-within-pair
                                nc.tensor.matmul(
                                    st[:, j * QS:(j + 1) * QS],
                                    kt[j * 64:(j + 1) * 64, pr, ki * KC:(ki + 1) * KC],
                                    qt[j * 64:(j + 1) * 64, pr, qs * QS:(qs + 1) * QS],
                                    start=True, stop=True,
                                )
                            t, hf = ki // 2, ki % 2
                            dve_pair = t in DVE_PAIRS
                            fp8_pair = FP8_AV and not dve_pair
                            if hf == 0:
                                # one er tile per ki-PAIR: [ki-half, head, q]
                                er = expp.tile(
                                    [128, 2, 2, QS],
                                    BF16 if not fp8_pair else FP8,
                                    tag="er", name=f"er_{qs}_{pr}_{t}")
                            if dve_pair:
                                # Schraudolph fast-exp on the vector engine:
                                # bits = int16(st*A + B); the int16 bit
                                # pattern read back as bf16 IS ~exp(st/8)
                                nc.vector.tensor_scalar(
                                    er[:, hf].bitcast(I16),
                                    st.rearrange("p (j q) -> p j q", j=2),
                                    FEXP_A, FEXP_B, Mult, Add,
                                )
                            else:
                                nc.scalar.activation(
                                    out=er[:, hf],
                                    in_=st.rearrange("p (j q) -> p j q", j=2),
                                    func=Exp, scale=0.125, bias=nbias,
                                )
                            if pr == 0 and qs == 0 and ki + 1 < NKC:
                                # pinned: V(s) one slot before PV(ki=s) reads it
                                emit_proj_v_one(ki + 1, psO)
                            if ki >= 1 or qs > 0 or pr > 0:
                                drip_budget += 2
                                while deferred and drip_budget >= deferred[0][0]:
                                    drip_budget -= deferred[0][0]
                                    deferred.pop(0)[1]()
                            if fp8_pair and hf == 1:
                                # DoubleRow: both k-chunks of the pair in one
                                # pass per head (fp8 weights [128, 2, 65])
                                for j in range(2):
                                    nc.tensor.matmul(
                                        ctx[j],
                                        va8[:, 2 * t:2 * t + 2, 2 * pr + j, 0:DH + 1],
                                        er[:, :, j, :],
                                        start=(t == 0), stop=(t == NKC // 2 - 1),
                                        perf_mode=DoubleRow,
                                    )
                            elif not fp8_pair:
                                for j in range(2):
                                    nc.tensor.matmul(
                                        ctx[j],
                                        va[:, ki, 2 * pr + j, :],
                                        er[:, hf, j, :],
                                        start=(not FP8_AV and ki == 0),
                                        stop=(not FP8_AV and ki == NKC - 1),
                                    )
                        for j in range(2):
                            emit_epilogue(ctx[j], pr, qs, j)
                        if pr == 1:
                            for m in range(8):
                                deferred.append((1, (
                                    lambda q_, m_, t_:
                                    lambda: emit_outproj_m(q_, m_, t_))
                                    (qs, m, qs == NQS - 1)))
                for _w, fn in deferred:
                    fn()

    nc.compile()
    return nc


def _shard_inputs(x, Wq, bq, Wk, bk, Wv, bv, Wo, bo):
    import ml_dtypes
    bf16 = ml_dtypes.bfloat16
    x = np.asarray(x, dtype=np.float32)
    in_maps = []
    for c in range(NCORES):
        b, g = c // 4, c % 4
        sl = slice(g * DL, (g + 1) * DL)
        in_maps.append({
            "XT": np.ascontiguousarray(x[b].T.astype(bf16)),
            "WQ": np.ascontiguousarray(np.asarray(Wq, np.float32)[:, sl].astype(bf16)),
            "WK": np.ascontiguousarray(np.asarray(Wk, np.float32)[:, sl].astype(bf16)),
            "WV": np.ascontiguousarray(np.asarray(Wv, np.float32)[:, sl].astype(bf16)),
            "WO": np.ascontiguousarray(np.asarray(Wo, np.float32)[sl, :].astype(bf16)),
            # [128, 2] columns: chunk m holds bias for d-range m*128..(m+1)*128
            "BQ": np.ascontiguousarray(
                np.asarray(bq, np.float32)[sl].reshape(2, 128).T),
            "BK": np.ascontiguousarray(
                np.asarray(bk, np.float32)[sl].reshape(2, 128).T),
        })
    return in_maps


def get_runner():
    global _RUNNER
    if _RUNNER is None:
        _RUNNER = _build_program()
    return _RUNNER


def kernel(x, Wq, bq, Wk, bk, Wv, bv, Wo, bo, **_ignored):
    from concourse.bass_utils import run_bass_kernel_spmd

    nc = get_runner()
    in_maps = _shard_inputs(x, Wq, bq, Wk, bk, Wv, bv, Wo, bo)
    res = run_bass_kernel_spmd(nc, in_maps, list(range(NCORES)))
    # bv never touches the device: softmax weights sum to 1, so V+bv adds
    # exactly bv to every normalized context row -> out += bv @ Wo.
    bias = (np.asarray(bv, np.float64) @ np.asarray(Wo, np.float64)
            + np.asarray(bo, np.float64))
    scale = 1.0 / WO8_SCALE if FP8_OUT else 1.0
    out = np.empty((B, S, D), dtype=np.float32)
    for b in range(B):
        acc = np.zeros((D, S), dtype=np.float64)
        for g in range(4):
            acc += np.asarray(res.results[4 * b + g]["OT"], np.float64)
        out[b] = (acc.T * scale + bias).astype(np.float32)
    return out


# revision 29
# speedup vs baseline: 1.0301x; 1.0043x over previous
"""Multi-head attention (B=2, S=2048, D=1024, H=16) on 8 TRN2 NeuronCores.

Sharding (data + tensor parallel, per the head-group hint):
  core c in 0..7 -> batch b = c // 4, head-group g = c % 4 (4 heads, 256 dims).
  Each core computes, for its batch and head group:
    QT = (x @ Wq_g + bq_g)^T          [256, 2048]   (d on partitions)
    KT likewise                       [256, 2048]
    V  = x @ Wv_g                     [2048, 256]   (S on partitions; bv folded
                                                     into the host-side output
                                                     constant: softmax weights
                                                     sum to 1, so +bv passes
                                                     through attention intact)
    per head h (4 local, Dh=64):
      ST_h = K_h @ Q_h^T              [2048k, 2048q] (scores transposed)
      E_h  = exp(ST_h / 8)            (softmax without max-subtraction; scores ~ N(0,1))
      CU_h = [V_h | 1]^T @ E_h        -> ctx^T unnormalized [64, q] + row of sums s_h[q]
      CT_h = CU_h / s_h               (ctx^T, normalized)
    OT_partial = Wo_g^T @ CT          [1024, 2048]  (out^T, partial over head groups)
  Host: out[b] = (sum_g OT_partial)^T + bo + bv @ Wo.

The whole data plane is bf16 (same precision class as the PE's fp32r path,
which rounds operands to ~bf16 anyway): the host pre-converts x^T and the
weights to bf16, halving input DMA, and the fp16 output halves output DMA.
The big tensors (xt/qt/kt/ct/va/weights) are double-buffered so consecutive
reps of the replicated timing program pipeline cleanly. Host passes x
pre-transposed per batch so no on-chip transpose of x is needed.

Engine budget: the PE stream (~137us: scores run as two 64-row-group matmuls
that the PE executes concurrently, AV+projections stream-bound) is the wall;
the exp stream costs ~(N+352)/1.2 ns per ACTIVATE => ~147us for all 128
tiles, so one ki-pair per segment is computed on the (otherwise slack)
vector engine with a Schraudolph fast-exp: one fused multiply-add into an
int16 view whose bit pattern IS ~exp(x) in bf16 (max ~3% per-element error,
which averages out through the softmax normalization), leaving ACT at
~128us. Q/K biases ride the existing psum->sbuf copies as per-partition
tensor_scalar adds, so the PE runs no bias matmuls at all. A softmax-
invariant shift keeps exp values small (also required by the disabled
fp8e4m3 DoubleRow AV experiment, which cost too much accuracy to enable).
"""

import numpy as np

B = 2
S = 2048
D = 1024
DL = 256          # local (per-core) d_model slice = 4 heads * 64
HL = 4            # local heads
DH = 64
QS = 512          # q tile (matmul free dim)
NQS = S // QS     # 4
KC = 128          # k chunk (psum partitions)
NKC = S // KC     # 16
DC = 128          # contraction chunk
NDC = D // DC     # 8
NCORES = 8

# Schraudolph fast-exp constants for exp(x * 0.125) via the bf16 bit pattern
# (top 16 bits of fp32): bits = int16(x * (2^7 * log2(e) / 8) + (127*2^7 - C)),
# C centers the piecewise-linear mantissa error to ~+-3%.
EXP_SHIFT = 2.25  # exp(st/8 - 2.25): softmax-invariant shift; max score ~8 -> e^5.75=314 < 448 (fp8e4m3 max), while minimizing subnormal-crushed weight mass
                  # exp values under fp8e4m3's 448 max (raw scores reach ~8)
FEXP_A = (1 << 7) * 1.4426950408889634 * 0.125
FEXP_B = (127.0 * (1 << 7) - 250829.0 / 65536.0
          - EXP_SHIFT * 1.4426950408889634 * (1 << 7))
# ki-PAIR slots of each (pr, qs) segment whose exp runs on DVE instead of ACT
# via the bf16 fast-exp (relieves the ACT wall: 128 exps = ~147us vs PE
# ~143us). With FP8_AV the non-DVE pairs would go fp8e4m3 + DoubleRow, but
# that path fails the accuracy gate and is disabled. Pairs 0 and 7 must stay
# on the ACT path (they carry the psum accumulation start/stop flags).
DVE_PAIRS = (4,)
FP8_AV = False
# fp8e4m3 DoubleRow out-projection: ct/wo quantize safely (no exp downstream,
# values well inside fp8 range); wo scaled x16 on-device to dodge subnormals,
# host divides the partials back.
FP8_OUT = False  # measured 3.4e-2 rel err (max-statistics of e4m3 noise) - fails the gate
WO8_SCALE = 16.0

_RUNNER = None


def _build_program(reps=1):
    import concourse.mybir as mybir
    import concourse.tile as tile
    from concourse import bacc

    F32 = mybir.dt.float32
    F32R = mybir.dt.float32r
    I16 = mybir.dt.int16
    BF16 = mybir.dt.bfloat16
    FP8 = mybir.dt.float8e4
    FP16 = mybir.dt.float16
    DoubleRow = mybir.MatmulPerfMode.DoubleRow
    Exp = mybir.ActivationFunctionType.Exp
    Mult = mybir.AluOpType.mult
    Add = mybir.AluOpType.add

    nc = bacc.Bacc("TRN2", target_bir_lowering=False, debug=False,
                   num_devices=NCORES, num_swdge_queues=4)

    XT = nc.dram_tensor("XT", [D, S], BF16, kind="ExternalInput").ap()
    WQ = nc.dram_tensor("WQ", [D, DL], BF16, kind="ExternalInput").ap()
    WK = nc.dram_tensor("WK", [D, DL], BF16, kind="ExternalInput").ap()
    WV = nc.dram_tensor("WV", [D, DL], BF16, kind="ExternalInput").ap()
    WO = nc.dram_tensor("WO", [DL, D], BF16, kind="ExternalInput").ap()
    # biases pre-shaped on host as [128, 2]: column m = bias for d-range
    # m*128..(m+1)*128 (per-partition scalars for the psum->sbuf copy)
    BQ = nc.dram_tensor("BQ", [128, 2], F32, kind="ExternalInput").ap()
    BK = nc.dram_tensor("BK", [128, 2], F32, kind="ExternalInput").ap()
    OT = nc.dram_tensor("OT", [D, S], FP16, kind="ExternalOutput").ap()

    with tile.TileContext(nc) as tc:
        with (
            tc.tile_pool(name="big", bufs=1) as big,      # long-lived tensors
            tc.tile_pool(name="exp", bufs=4) as expp,     # exp(ST) tiles
            tc.tile_pool(name="outc", bufs=3) as outc,    # out-proj copies
            tc.tile_pool(name="misc", bufs=3) as misc,    # recip rows etc
            tc.tile_pool(name="psA", bufs=2, space="PSUM") as psA,
            tc.tile_pool(name="psC", bufs=3, space="PSUM") as psC,
            tc.tile_pool(name="psO", bufs=1, space="PSUM") as psO,
        ):
            for _rep in range(reps):
                # ---- load inputs (DMA casts fp32 -> fp32r on the fly) ----
                # Inputs are spread across the HWDGE queues of the engines
                # that are idle at startup (SP / ACT / DVE) so descriptor
                # generation doesn't serialize on one queue: first matmul
                # needs wk + xt c0..3, first exp needs all of xt + wk + wq.
                # wv lands right after xt, wo (out-proj, ~100us in) last.
                bqs = big.tile([128, 2], F32, tag="bqs", bufs=2)
                bks = big.tile([128, 2], F32, tag="bks", bufs=2)
                nc.gpsimd.dma_start(out=bqs, in_=BQ)
                nc.gpsimd.dma_start(out=bks, in_=BK)
                wq = big.tile([128, NDC, DL], BF16, tag="wq", bufs=2)
                wk = big.tile([128, NDC, DL], BF16, tag="wk", bufs=2)
                wv = big.tile([128, NDC, DL], BF16, tag="wv", bufs=2)
                nc.scalar.dma_start(
                    out=wk,
                    in_=WK.rearrange("(c p) n -> p c n", p=128))
                nc.sync.dma_start(
                    out=wq,
                    in_=WQ.rearrange("(c p) n -> p c n", p=128))
                xt = big.tile([128, NDC, S], BF16, tag="xt", bufs=2)   # x^T
                xt_src = XT.rearrange("(c p) q -> p c q", p=128)
                xt_eng = (nc.sync, nc.scalar)
                for c in range(NDC):
                    xt_eng[c % 2].dma_start(out=xt[:, c, :], in_=xt_src[:, c, :])
                # wv/wo at the tail of the two HWDGE queues so they can't
                # jump ahead of the xt stream on the DMA engines; wo is
                # needed only by the out-projection (~100us in)
                nc.sync.dma_start(
                    out=wv,
                    in_=WV.rearrange("(c p) n -> p c n", p=128)
                )
                wo = big.tile([128, 2, D], BF16, tag="wo", bufs=2)
                nc.scalar.dma_start(
                    out=wo,
                    in_=WO.rearrange("(c p) n -> p c n", p=128))
                wo8 = None
                if FP8_OUT:
                    wo8 = big.tile([128, 2, D], FP8, tag="wo8", bufs=2)
                    nc.gpsimd.tensor_scalar_mul(wo8, wo, WO8_SCALE)

                onesc_f = big.tile([128, HL, 1], BF16, tag="onesc_f", bufs=2)
                nc.vector.memset(onesc_f, 1.0)
                nbias = big.tile([128, 1], F32, tag="nbias", bufs=2)
                nc.vector.memset(nbias, -EXP_SHIFT)

                # ---- projections ----
                # Only the minimal prefix is emitted up-front; all other
                # projection work drips into the attention loop so its PE
                # time hides under the exp stream.
                qt = big.tile([128, 2, S], BF16, tag="qt", bufs=2)  # Q^T: [d(2x128), q]
                kt = big.tile([128, 2, S], BF16, tag="kt", bufs=2)
                va = big.tile([128, NKC, HL, DH + 1], BF16, tag="va", bufs=2)
                # the softmax-sum ones column, for all k-chunks at once
                nc.gpsimd.memset(va[:, :, :, DH:DH + 1], 1.0)
                # fp8 copy of [V | 1] for the DoubleRow AV path; last dim
                # padded to 68 so the ki-pair stride (4*68 = 272B) is 16B-
                # aligned as DoubleRow weight APs require
                va8 = None
                if FP8_AV:
                    va8 = big.tile([128, NKC, HL, 68], FP8, tag="va8")
                    # DoubleRow weight reads are 16B-granular: zero the pad
                    # columns so they can never inject garbage
                    nc.gpsimd.memset(va8, 0.0)

                def emit_proj_qk_one(m, qs, w_t, bcol, dst, pool):
                    # tag shares slots with the pool's other tiles (time-disjoint)
                    p = pool.tile([128, QS], F32,
                                  tag="mm" if pool is psA else "op",
                                  name=f"pj_{m}_{qs}_{dst.tensor.name}")
                    for c in range(NDC):
                        nc.tensor.matmul(
                            p,
                            w_t[:, c, m * 128:(m + 1) * 128],
                            xt[:, c, qs * QS:(qs + 1) * QS],
                            start=(c == 0),
                            stop=(c == NDC - 1),
                        )
                    # psum->sbuf copy + bias add in one DVE tensor_scalar
                    nc.vector.tensor_scalar_add(
                        dst[:, m, qs * QS:(qs + 1) * QS], p, bcol[:, m:m + 1],
                    )

                def emit_proj_qk(m, pool):
                    # KT first: scores need all of K^T but only one q-slice of Q^T
                    for w_t, bcol, dst in ((wk, bks, kt), (wq, bqs, qt)):
                        for qs in range(NQS):
                            emit_proj_qk_one(m, qs, w_t, bcol, dst, pool)

                def emit_proj_v_one(sc, pool):
                    # V in [S, d] layout, augmented with a ones column per head
                    p = pool.tile([128, DL], F32,
                                  tag="mm" if pool is psA else "op",
                                  name=f"pv_{sc}")
                    for c in range(NDC):
                        nc.tensor.matmul(
                            p,
                            xt[:, c, sc * 128:(sc + 1) * 128],
                            wv[:, c, :],
                            start=(c == 0),
                            stop=(c == NDC - 1),
                        )
                    nc.vector.tensor_copy(
                        out=va[:, sc, :, 0:DH],
                        in_=p.rearrange("p (h d) -> p h d", h=HL),
                    )
                    if FP8_AV:
                        nc.vector.tensor_copy(
                            out=va8[:, sc, :, 0:DH],
                            in_=p.rearrange("p (h d) -> p h d", h=HL),
                        )
                        nc.gpsimd.tensor_copy(
                            out=va8[:, sc, :, DH:DH + 1], in_=onesc_f
                        )

                def emit_proj_qk_split(m, qs, w_t, bcol, dst):
                    # contraction split in half so the first 4 matmuls start
                    # when xt chunk 3 lands (~half the input-DMA time)
                    p1 = psA.tile([128, QS], F32, tag="mm",
                                  name=f"pjA_{m}_{qs}_{dst.tensor.name}")
                    for c in range(4):
                        nc.tensor.matmul(
                            p1, w_t[:, c, m * 128:(m + 1) * 128],
                            xt[:, c, qs * QS:(qs + 1) * QS],
                            start=(c == 0), stop=(c == 3),
                        )
                    s1 = misc.tile([128, QS], F32, tag="s1",
                                   name=f"s1_{m}_{qs}_{dst.tensor.name}")
                    # fold the bias into the first-half copy
                    nc.vector.tensor_scalar_add(s1, p1, bcol[:, m:m + 1])
                    p2 = psA.tile([128, QS], F32, tag="mm",
                                  name=f"pjB_{m}_{qs}_{dst.tensor.name}")
                    for c in range(4, NDC):
                        nc.tensor.matmul(
                            p2, w_t[:, c, m * 128:(m + 1) * 128],
                            xt[:, c, qs * QS:(qs + 1) * QS],
                            start=(c == 4), stop=(c == NDC - 1),
                        )
                    nc.vector.tensor_tensor(
                        out=dst[:, m, qs * QS:(qs + 1) * QS],
                        in0=p2, in1=s1, op=Add,
                    )

                # prefix: the minimum needed for attention (qs=0, pr=0, ki<4):
                # K^T(m0, k 0:512), Q^T(m0, q 0:512), V(s 0:128)
                emit_proj_qk_split(0, 0, wk, bks, kt)
                emit_proj_qk_split(0, 0, wq, bqs, qt)
                emit_proj_v_one(0, psO)
                # everything else drips into the attention stream below

                # ---- attention, epilogue, out-projection ----
                ct = big.tile([128, 2, S], FP8 if FP8_OUT else BF16,
                              tag="ct", bufs=2)   # ctx^T: [d(2x128), q]

                def emit_epilogue(ctx_j, pr, qs, j):
                    # normalize ctx^T by the softmax denominator (psum row 64),
                    # PE-free: DVE recip -> gpsimd partition_broadcast -> DVE mult
                    rc = misc.tile([1, QS], F32, tag="rc", name=f"rc_{qs}_{pr}_{j}")
                    nc.vector.reciprocal(out=rc, in_=ctx_j[DH:DH + 1, :])
                    bc = misc.tile([64, QS], F32, tag="bc", name=f"bc_{qs}_{pr}_{j}")
                    nc.gpsimd.partition_broadcast(bc, rc, channels=64)
                    nc.vector.tensor_tensor(
                        out=ct[j * 64:(j + 1) * 64, pr, qs * QS:(qs + 1) * QS],
                        in0=ctx_j[0:DH, :],
                        in1=bc,
                        op=Mult,
                    )

                def emit_outproj_m(qs, m, tail=False):
                    p = psO.tile([128, QS], F32, tag="op", name=f"op_{qs}_{m}")
                    if FP8_OUT:
                        # both 128-row contraction chunks in one DoubleRow pass
                        nc.tensor.matmul(
                            p,
                            wo8[:, :, m * 128:(m + 1) * 128],
                            ct[:, :, qs * QS:(qs + 1) * QS],
                            start=True, stop=True,
                            perf_mode=DoubleRow,
                        )
                    else:
                        for c in range(2):
                            nc.tensor.matmul(
                                p,
                                wo[:, c, m * 128:(m + 1) * 128],
                                ct[:, c, qs * QS:(qs + 1) * QS],
                                start=(c == 0), stop=(c == 1),
                            )
                    o = outc.tile([128, QS], FP16, tag="o", name=f"o_{qs}_{m}")
                    if tail:
                        # rep tail: ACT is idle after the last exp; putting the
                        # psum->sbuf copies there frees the psO slot without
                        # waiting on the DVE queue
                        nc.scalar.copy(out=o, in_=p)
                    else:
                        nc.vector.tensor_copy(out=o, in_=p)
                    nc.sync.dma_start(
                        out=OT[m * 128:(m + 1) * 128, qs * QS:(qs + 1) * QS],
                        in_=o,
                    )

                # drip queue: closures emitted a-few-per-ki inside the attention
                # segments so their PE work overlaps the ACT-bound exp stream.
                # Order respects first-consumer: KT(m0,s) before scores reach
                # k=s*512; V(s) before PV(ki=s); QT(m0,s) before segment qs=s;
                # chunk-1 Q/K before the pr=1 phase; out-proj appended later.
                deferred = []   # (weight, closure): weight ~ PE ki-slots (x2)
                def _defer_qk(m_, qs_, w_, b_, d_, pool_, wt):
                    deferred.append((wt, (
                        lambda a, b, c, dd, p_:
                        lambda: emit_proj_qk_one(m_, a, b, c, dd, p_))
                        (qs_, w_, b_, d_, pool_)))

                def _defer_v(sc_):
                    deferred.append((2, (
                        lambda s_: lambda: emit_proj_v_one(s_, psO))(sc_)))

                # NOTE: Tile dependencies are emission-order-based — every
                # producer MUST be emitted before its first consumer. V(s) is
                # pinned at slot s-1 of segment (0,0) (not queued); KT(m0,s)
                # pops by slot s (scores need it from ki=4s); QT(m0,s) pops
                # well before segment (0,s).
                _defer_qk(0, 1, wk, bks, kt, psA, 2)
                _defer_qk(0, 2, wk, bks, kt, psA, 2)
                _defer_qk(0, 3, wk, bks, kt, psA, 2)
                _defer_qk(0, 1, wq, bqs, qt, psA, 2)
                _defer_qk(0, 2, wq, bqs, qt, psA, 2)
                _defer_qk(0, 3, wq, bqs, qt, psA, 2)
                for qs_ in range(NQS):
                    for w_, b_, d_ in ((wk, bks, kt), (wq, bqs, qt)):
                        _defer_qk(1, qs_, w_, b_, d_, psO, 4)
                drip_budget = 0
                for pr in range(2):              # head pair = (2pr, 2pr+1)
                    for qs in range(NQS):
                        ctx = [
                            psC.tile([DH + 1, QS], F32, tag="ctx",
                                     name=f"ctx_{qs}_{pr}_{j}")
                            for j in range(2)
                        ]
                        er = None
                        for ki in range(NKC):
                            st = psA.tile([128, 2 * QS], F32, tag="mm",
                                          name=f"st_{qs}_{pr}_{ki}")
                            for j in range(2):   # j: head-within-pair
                                nc.tensor.matmul(
                                    st[:, j * QS:(j + 1) * QS],
                                    kt[j * 64:(j + 1) * 64, pr, ki * KC:(ki + 1) * KC],
                                    qt[j * 64:(j + 1) * 64, pr, qs * QS:(qs + 1) * QS],
                                    start=True, stop=True,
                                )
                            t, hf = ki // 2, ki % 2
                            dve_pair = t in DVE_PAIRS
                            fp8_pair = FP8_AV and not dve_pair
                            if hf == 0:
                                # one er tile per ki-PAIR: [ki-half, head, q]
                                er = expp.tile(
                                    [128, 2, 2, QS],
                                    BF16 if not fp8_pair else FP8,
                                    tag="er", name=f"er_{qs}_{pr}_{t}")
                            if dve_pair:
                                # Schraudolph fast-exp on the vector engine:
                                # bits = int16(st*A + B); the int16 bit
                                # pattern read back as bf16 IS ~exp(st/8)
                                nc.vector.tensor_scalar(
                                    er[:, hf].bitcast(I16),
                                    st.rearrange("p (j q) -> p j q", j=2),
                                    FEXP_A, FEXP_B, Mult, Add,
                                )
                            else:
                                nc.scalar.activation(
                                    out=er[:, hf],
                                    in_=st.rearrange("p (j q) -> p j q", j=2),
                                    func=Exp, scale=0.125, bias=nbias,
                                )
                            if pr == 0 and qs == 0 and ki + 1 < NKC:
                                # pinned: V(s) one slot before PV(ki=s) reads it
                                emit_proj_v_one(ki + 1, psO)
                            if ki >= 1 or qs > 0 or pr > 0:
                                drip_budget += 2
                                while deferred and drip_budget >= deferred[0][0]:
                                    drip_budget -= deferred[0][0]
                                    deferred.pop(0)[1]()
                            if fp8_pair and hf == 1:
                                # DoubleRow: both k-chunks of the pair in one
                                # pass per head (fp8 weights [128, 2, 65])
                                for j in range(2):
                                    nc.tensor.matmul(
                                        ctx[j],
                                        va8[:, 2 * t:2 * t + 2, 2 * pr + j, 0:DH + 1],
                                        er[:, :, j, :],
                                        start=(t == 0), stop=(t == NKC // 2 - 1),
                                        perf_mode=DoubleRow,
                                    )
                            elif not fp8_pair:
                                for j in range(2):
                                    nc.tensor.matmul(
                                        ctx[j],
                                        va[:, ki, 2 * pr + j, :],
                                        er[:, hf, j, :],
                                        start=(not FP8_AV and ki == 0),
                                        stop=(not FP8_AV and ki == NKC - 1),
                                    )
                        for j in range(2):
                            emit_epilogue(ctx[j], pr, qs, j)
                        if pr == 1:
                            for m in range(8):
                                deferred.append((1, (
                                    lambda q_, m_, t_:
                                    lambda: emit_outproj_m(q_, m_, t_))
                                    (qs, m, qs == NQS - 1)))
                for _w, fn in deferred:
                    fn()

    nc.compile()
    return nc


def _shard_inputs(x, Wq, bq, Wk, bk, Wv, bv, Wo, bo):
    import ml_dtypes
    bf16 = ml_dtypes.bfloat16
    x = np.asarray(x, dtype=np.float32)
    in_maps = []
    for c in range(NCORES):
        b, g = c // 4, c % 4
        sl = slice(g * DL, (g + 1) * DL)
        in_maps.append({
            "XT": np.ascontiguousarray(x[b].T.astype(bf16)),
            "WQ": np.ascontiguousarray(np.asarray(Wq, np.float32)[:, sl].astype(bf16)),
            "WK": np.ascontiguousarray(np.asarray(Wk, np.float32)[:, sl].astype(bf16)),
            "WV": np.ascontiguousarray(np.asarray(Wv, np.float32)[:, sl].astype(bf16)),
            "WO": np.ascontiguousarray(np.asarray(Wo, np.float32)[sl, :].astype(bf16)),
            # [128, 2] columns: chunk m holds bias for d-range m*128..(m+1)*128
            "BQ": np.ascontiguousarray(
                np.asarray(bq, np.float32)[sl].reshape(2, 128).T),
            "BK": np.ascontiguousarray(
                np.asarray(bk, np.float32)[sl].reshape(2, 128).T),
        })
    return in_maps


def get_runner():
    global _RUNNER
    if _RUNNER is None:
        _RUNNER = _build_program()
    return _RUNNER


def kernel(x, Wq, bq, Wk, bk, Wv, bv, Wo, bo, **_ignored):
    from concourse.bass_utils import run_bass_kernel_spmd

    nc = get_runner()
    in_maps = _shard_inputs(x, Wq, bq, Wk, bk, Wv, bv, Wo, bo)
    res = run_bass_kernel_spmd(nc, in_maps, list(range(NCORES)))
    # bv never touches the device: softmax weights sum to 1, so V+bv adds
    # exactly bv to every normalized context row -> out += bv @ Wo.
    bias = (np.asarray(bv, np.float64) @ np.asarray(Wo, np.float64)
            + np.asarray(bo, np.float64))
    scale = 1.0 / WO8_SCALE if FP8_OUT else 1.0
    out = np.empty((B, S, D), dtype=np.float32)
    for b in range(B):
        acc = np.zeros((D, S), dtype=np.float64)
        for g in range(4):
            acc += np.asarray(res.results[4 * b + g]["OT"], np.float64)
        out[b] = (acc.T * scale + bias).astype(np.float32)
    return out
